# revision 1
# baseline (speedup 1.0000x reference)
"""Trainium2 Bass kernel for the SNN (two-layer LIF, snnTorch-style) problem.

Math (per batch row b, fp32):
    cur1 = x @ W1.T + b1                       # [B, NH], constant across steps
    mem1_{t+1} = beta*mem1_t + cur1 - H(mem1_t - 1)      (mem1_1 = cur1)
    spk1_t  = H(mem1_t - 1)                    # == reset used at step t+1
    cur2_t  = spk1_t @ W2.T + b2
    mem2_t  = beta*mem2_{t-1} + cur2_t - H(mem2_{t-1} - 1)
    outputs: mem2_rec[t] = mem2_t, spk2_rec[t] = H(mem2_t - 1)

Key kernel ideas:
  * spk1 is never materialized: since s_t = beta*mem_t + cur1 - mem_{t+1}
    (exactly, up to fp32 rounding), cur2_t = beta*r_t - r_{t+1} + q with
    r_t = W2 @ mem_t and q = W2 @ cur1 + b2.
  * r_t is computed with the *state as the stationary matmul operand*:
    out[128b, 2] = memblock[128h, 128b].T @ W2chunk[128h, 2].  The moving
    free size is only NO=2 rows, so the fp32 4-cycles/row tensor-engine
    penalty is negligible; the naive form streams 16K rows/step.
    Accumulation chains run j-outer/i-inner (chains must be sequential),
    split into two PSUM halves (tiles 0-3 / 4-7) so the first half runs
    while the DVE is still updating later tiles; a single DVE op adds the
    halves into rbuf.
  * The elementwise LIF update is split across engines per step: 7 h-tiles
    on DVE via a fused custom op (one 1x pass each), 1 h-tile on
    Pool/GPSIMD via the ops its ISA allows (tensor_scalar compare/scale
    with immediates + tensor_tensor add/sub) with the beta*mem scale done
    on the otherwise-idle Act engine.  Rounding order matches the
    reference: (beta*mem + cur1) - (mem > 1).
  * The tiny per-step tail (cur2/spk2 on Pool, mem2 LIF on DVE) is
    software-pipelined one step behind the big tiles so it never stalls
    the DVE; outputs accumulate in SBUF and ship in two end-of-kernel
    DMAs with per-partition-contiguous layout (host does final reshape).

Data parallel over batch: 16384 rows -> 8 cores x 2048.
"""

import sys

if "/opt/trn_rl_repo" not in sys.path:
    sys.path.insert(0, "/opt/trn_rl_repo")

import numpy as np

import concourse.bacc as bacc
import concourse.bass as bass
import concourse.mybir as mybir
import concourse.tile as tile
from concourse.bass_utils import run_bass_kernel_spmd

# Problem constants (hardcoded; kernel.py must be self-contained).
B, NI, NH, NO, T = 16384, 100, 1000, 2, 25
NCORES = 8
BS = B // NCORES          # 2048 batch rows per core
NHP = 1024                # hidden padded to 8 * 128
NT = NHP // 128           # 8 hidden tiles
NDVE = 7                  # tiles 0..6 updated on DVE (fused custom op)
DVE_GROUPS = ((0, 2), (2, 4), (4, 6), (6, 7))  # piece spans
DVE_MERGED = ((0, 2), (2, 5))  # pieces merged per custom op: tiles 0-3, 4-6
NA = 4                    # tiles 0..3 accumulate into PSUM half A
SP6 = 256                 # batch columns of tile 6 updated on Pool
C6 = BS - SP6             # DVE's share of tile 6
SETUP_ORDER = (7, 0, 1, 2, 3, 4, 5, 6)  # pool tile's cur1 first
NBLK = BS // 128          # 16 batch blocks
NCH = BS // 512           # 4 moving chunks of 512 (setup cur1 matmuls)
BETA = 0.95
THR = 1.0
F32 = mybir.dt.float32
AOP = mybir.AluOpType
AFT = mybir.ActivationFunctionType

_LIF_OP = None


def _get_lif_op():
    """Register (once) the fused LIF-step op: out = (in0*s0 + in1) - (in0 > s1)."""
    global _LIF_OP
    if _LIF_OP is not None:
        return _LIF_OP
    from concourse import dve_ops
    from concourse.dve_spec import Spec, Src0, Src1, C0, C1, lower, _has_src1
    from concourse.dve_uop import DveOpSpec

    name = "LIF_STEP_ANT"
    for op in dve_ops.OPS:
        if op.name == name:
            _LIF_OP = op
            return op

    spec = Spec(
        body=(Src0 * C0 + Src1) - (Src0 > C1),
        reference=lambda in0, in1, s0, s1, imm2: (
            in0.astype(np.float32) * s0 + in1
        )
        - (in0 > s1).astype(np.float32),
    )
    row = dve_ops._CUSTOM_DVE_ROW_BASE + len(dve_ops.OPS)
    assert row < 0x20, "custom-DVE row space exhausted"
    dve_ops._SUB_OPCODE_FOR_NAME[name] = row
    shas = {}
    for ver in ("v3", "v4"):
        s = DveOpSpec(
            name=name, opcode=row, uops=lower(spec, ver=ver), rd1_en=_has_src1(spec)
        )
        shas[ver] = s.sha(ver)
    op = dve_ops.DveOp(name, spec, subdim=False, uops_sha=shas)
    dve_ops.OPS.append(op)
    dve_ops.CUSTOM_DVE_SPECS[name] = spec
    _LIF_OP = op
    return op


# Packed-input layout (one dram param; the small weights piece is DMA'd
# first so setup matmuls start early, then x.T streams in 4 chunked DMAs
# into a setup-scoped tile).
OFF_W1 = 0                      # [:NI, NHP] W1.T (padded)
OFF_B1 = OFF_W1 + NHP           # [128, NT] b1 per-tile columns
OFF_W2 = OFF_B1 + NT            # [128, NT*NO] W2.T tiles
OFF_B2 = OFF_W2 + NT * NO       # [128, NBLK*NO] b2 broadcast (blk,o)
OFF_ID = OFF_B2 + NBLK * NO     # [128, 128] identity (PE warmups)
OFF_XT = OFF_ID + 128           # [:NI, BS] x.T (host-transposed)
BLOBF = OFF_XT + BS


def _build_program():
    lif = _get_lif_op()

    nc = bacc.Bacc(
        "TRN2",
        target_bir_lowering=False,
        debug=False,
        num_devices=NCORES,
    )
    blob_d = nc.declare_dram_parameter("blob", [128, BLOBF], F32, isOutput=False)
    # Outputs in per-partition-contiguous layout [128, T*NBLK*NO]; host
    # reassembles to [T, BS, NO].
    m2_d = nc.declare_dram_parameter("m2", [128, T * NBLK * NO], F32, isOutput=True)
    s2_d = nc.declare_dram_parameter("s2", [128, T * NBLK * NO], F32, isOutput=True)

    with tile.TileContext(nc) as tc:
        with (
            tc.tile_pool(name="const", bufs=1) as constp,
            tc.tile_pool(name="state", bufs=1) as statep,
        ):
            blob = constp.tile([128, OFF_XT], F32)
            nc.sync.dma_start(blob[:], blob_d[:, :OFF_XT])
            w1t = blob[:NI, OFF_W1 : OFF_W1 + NHP]
            b1c = blob[:, OFF_B1 : OFF_B1 + NT]
            w2s = blob[:, OFF_W2 : OFF_W2 + NT * NO].rearrange(
                "p (i o) -> p i o", o=NO
            )
            b2t = blob[:, OFF_B2 : OFF_B2 + NBLK * NO]
            idn = blob[:, OFF_ID : OFF_ID + 128]

            cur1 = statep.tile([128, NT, BS], F32)
            mem = statep.tile([128, NT, BS], F32)
            rbuf = statep.tile([128, T + 2, NBLK * NO], F32)  # r_t, slots 1..T+1
            q_sb = statep.tile([128, NBLK * NO], F32)
            cur2 = statep.tile([128, NBLK * NO], F32)
            btmp = statep.tile([128, NBLK * NO], F32)
            m2rec = statep.tile([128, T, NBLK * NO], F32)
            s2rec = statep.tile([128, T, NBLK * NO], F32)
            zer32 = constp.tile([128, NBLK * NO], F32)
            nc.vector.memset(zer32[:], 0.0)

            # One-time per-engine "touch" of the blob so the DMA-completion
            # wait is observed once per engine.
            scr = constp.tile([1, 3], F32)
            nc.scalar.activation(scr[:, 0:1], blob[:1, 0:1], AFT.Copy)
            nc.vector.tensor_copy(scr[:, 1:2], blob[:1, 0:1])
            nc.gpsimd.tensor_copy(scr[:, 2:3], blob[:1, 0:1])

            # ---- setup: cur1 tiles = W1_i @ x.T + b1_i (Act adds bias);
            # x.T lives in a setup-scoped tile so its SBUF is reusable. ----
            with (
                tc.tile_pool(name="xp", bufs=1) as xp,
                tc.tile_pool(name="pss", bufs=4, space=bass.MemorySpace.PSUM) as pss,
                tc.tile_pool(name="pw", bufs=1, space=bass.MemorySpace.PSUM) as pwp,
            ):
                xt = xp.tile([128, BS], F32)
                for c in range(NCH):
                    nc.sync.dma_start(
                        xt[:, c * 512 : (c + 1) * 512],
                        blob_d[:, OFF_XT + c * 512 : OFF_XT + (c + 1) * 512],
                    )
                # Tiny warm-up matmuls: soak the tensor engine's cold
                # p-state slots (the cost model prices p-state at dispatch)
                # so the real 512-row matmuls below run at full clock.
                warm = pwp.tile([2, 2], F32)
                for _ in range(16):
                    nc.tensor.matmul(warm[:], idn[:, 0:2], idn[:, 0:2])
                for i in SETUP_ORDER:
                    for c in range(NCH):
                        pc = pss.tile([128, 512], F32, tag="pc")
                        nc.tensor.matmul(
                            pc[:],
                            w1t[:, i * 128 : (i + 1) * 128],
                            xt[:NI, c * 512 : (c + 1) * 512],
                        )
                        nc.scalar.activation(
                            cur1[:, i, c * 512 : (c + 1) * 512],
                            pc[:],
                            AFT.Identity,
                            bias=b1c[:, i : i + 1],
                        )

            gbuf = statep.tile([128, BS], F32)           # Act: beta*mem (tile 7)
            spl = statep.tile([128, BS], F32)            # Pool: spike (tile 7)
            g6 = statep.tile([128, SP6], F32)            # Pool: tile-6 slice
            s6 = statep.tile([128, SP6], F32)
            sbar = statep.tile([128, BS], F32)           # DVE: -spike (tile 6)

            with tc.tile_pool(
                name="pr", bufs=2, space=bass.MemorySpace.PSUM
            ) as prp:

                def r_halves(src, tag):
                    """Two PSUM halves: A = sum_{i<NA} src_i.T @ W2_i,
                    B = sum_{i>=NA}.  Chains are j-outer/i-inner (sequential
                    per output region); state block is stationary."""
                    pa = prp.tile([128, NBLK * NO], F32, tag=tag + "a")
                    pb = prp.tile([128, NBLK * NO], F32, tag=tag + "b")
                    for p, lo, hi in ((pa, 0, NA), (pb, NA, NT)):
                        for j in range(NBLK):
                            for i in range(lo, hi):
                                nc.tensor.matmul(
                                    p[:, j * NO : (j + 1) * NO],
                                    src[:, i, j * 128 : (j + 1) * 128],
                                    w2s[:, i, :],
                                    start=(i == lo),
                                    stop=(i == hi - 1),
                                )
                    return pa, pb

                # ---- main loop: iteration k makes mem_k and r_k; the tail
                # for step t = k-2 is pipelined in (cur2/spk2 on Pool, mem2
                # on DVE), so it never stalls the big tile updates. ----
                def pool_tail(t):
                    """cur2, spk2(t-1) and mem2(t) for tail step t, all on
                    Pool, in the reference rounding order.  spk2 of step
                    t-1 doubles as mem2's reset (detached, same value)."""
                    nc.gpsimd.tensor_scalar(
                        cur2[:], rbuf[:, t, :], BETA, None, AOP.mult
                    )
                    nc.gpsimd.tensor_tensor(
                        cur2[:], cur2[:], rbuf[:, t + 1, :], AOP.subtract
                    )
                    nc.gpsimd.tensor_tensor(cur2[:], cur2[:], q_sb[:], AOP.add)
                    if t >= 2:
                        nc.gpsimd.tensor_scalar(
                            s2rec[:, t - 2, :], m2rec[:, t - 2, :], THR, None,
                            AOP.is_gt,
                        )
                    prev = zer32 if t == 1 else m2rec[:, t - 2, :]
                    rst = zer32 if t == 1 else s2rec[:, t - 2, :]
                    nc.gpsimd.tensor_scalar(
                        m2rec[:, t - 1, :], prev[:], BETA, None, AOP.mult
                    )
                    nc.gpsimd.tensor_tensor(
                        m2rec[:, t - 1, :], m2rec[:, t - 1, :], cur2[:], AOP.add
                    )
                    nc.gpsimd.tensor_tensor(
                        m2rec[:, t - 1, :], m2rec[:, t - 1, :], rst[:],
                        AOP.subtract,
                    )

                for k in range(2, T + 2):
                    t = k - 2  # tail step emitted this iteration (0 = none)
                    if t >= 1:
                        pool_tail(t)

                    src = cur1 if k == 2 else mem
                    # Pool tile 7 (Act supplies beta*mem; Pool does the rest
                    # in the reference rounding order (g + cur1) - s).  At
                    # k=2 work chunk-by-chunk as cur1 lands from setup.
                    i7 = NT - 1
                    csl = (
                        [slice(c * 512, (c + 1) * 512) for c in range(NCH)]
                        if k == 2
                        else [slice(None)]
                    )
                    for sl in csl:
                        nc.scalar.activation(
                            gbuf[:, sl], src[:, i7, sl], AFT.Copy, scale=BETA
                        )
                        nc.gpsimd.tensor_scalar(
                            spl[:, sl], src[:, i7, sl], THR, None, AOP.is_gt
                        )
                        nc.gpsimd.tensor_tensor(
                            gbuf[:, sl], gbuf[:, sl], cur1[:, i7, sl], AOP.add
                        )
                        nc.gpsimd.tensor_tensor(
                            mem[:, i7, sl], gbuf[:, sl], spl[:, sl],
                            AOP.subtract,
                        )
                    # DVE tiles: fused custom op.  k=2 runs per cur1 chunk so
                    # the first op starts as soon as setup's first chunk
                    # lands; later iterations use two merged passes.
                    if k == 2:
                        for i in range(NDVE):
                            for sl in csl:
                                nc.vector._custom_dve(
                                    lif,
                                    out=mem[:, i, sl],
                                    in0=src[:, i, sl],
                                    in1=cur1[:, i, sl],
                                    s0=BETA,
                                    s1=THR,
                                )
                    else:
                        # Pool also takes the last SP6 batch columns of
                        # tile 6 (pure elementwise split, no cross reads).
                        nc.gpsimd.tensor_scalar(
                            s6[:], src[:, 6, C6:], THR, None, AOP.is_gt
                        )
                        nc.gpsimd.tensor_scalar(
                            g6[:], src[:, 6, C6:], BETA, None, AOP.mult
                        )
                        nc.gpsimd.tensor_tensor(
                            g6[:], g6[:], cur1[:, 6, C6:], AOP.add
                        )
                        nc.gpsimd.tensor_tensor(
                            mem[:, 6, C6:], g6[:], s6[:], AOP.subtract
                        )
                        for lo, hi in ((0, NA), (NA, 6)):
                            nc.vector._custom_dve(
                                lif,
                                out=mem[:, lo:hi, :],
                                in0=src[:, lo:hi, :],
                                in1=cur1[:, lo:hi, :],
                                s0=BETA,
                                s1=THR,
                            )
                        nc.vector._custom_dve(
                            lif,
                            out=mem[:, 6, :C6],
                            in0=src[:, 6, :C6],
                            in1=cur1[:, 6, :C6],
                            s0=BETA,
                            s1=THR,
                        )

                    if k == 2:
                        # r_1 and q from cur1 (mem_1 = cur1); emitted after
                        # the k=2 state updates so the in-order DVE queue is
                        # not blocked behind the full cur1 production.
                        pa, pb = r_halves(cur1, "pr")
                        nc.scalar.activation(rbuf[:, 1, :], pa[:], AFT.Copy)
                        nc.scalar.activation(btmp[:], pb[:], AFT.Copy)
                        nc.gpsimd.tensor_tensor(
                            rbuf[:, 1, :], rbuf[:, 1, :], btmp[:], AOP.add
                        )
                        nc.gpsimd.tensor_tensor(
                            q_sb[:], rbuf[:, 1, :], b2t[:], AOP.add
                        )

                    pa, pb = r_halves(mem, "pr")
                    nc.scalar.activation(rbuf[:, k, :], pa[:], AFT.Copy)
                    nc.scalar.activation(btmp[:], pb[:], AFT.Copy)
                    nc.gpsimd.tensor_tensor(
                        rbuf[:, k, :], rbuf[:, k, :], btmp[:], AOP.add
                    )

                # ---- epilogue: tail step t = T and trailing spk2. ----
                pool_tail(T)
                nc.gpsimd.tensor_scalar(
                    s2rec[:, T - 2, :], m2rec[:, T - 2, :], THR, None, AOP.is_gt
                )
                nc.gpsimd.tensor_scalar(
                    s2rec[:, T - 1, :], m2rec[:, T - 1, :], THR, None, AOP.is_gt
                )

            nc.sync.dma_start(
                m2_d[:].rearrange("p (t f) -> p t f", f=NBLK * NO),
                m2rec[:, :, :],
            )
            nc.sync.dma_start(
                s2_d[:].rearrange("p (t f) -> p t f", f=NBLK * NO),
                s2rec[:, :, :],
            )
    nc.compile()
    return nc


_PROG = None


def _get_prog():
    global _PROG
    if _PROG is None:
        _PROG = _build_program()
    return _PROG


def _make_in_maps(x, W1, b1, W2, b2):
    x = np.ascontiguousarray(np.asarray(x, np.float32))
    W1 = np.asarray(W1, np.float32)
    b1 = np.asarray(b1, np.float32)
    W2 = np.asarray(W2, np.float32)
    b2 = np.asarray(b2, np.float32)

    w1t = np.zeros((NI, NHP), np.float32)
    w1t[:, :NH] = W1.T
    b1p = np.zeros((NHP,), np.float32)
    b1p[:NH] = b1
    b1c = b1p.reshape(NT, 128).T  # [128, NT]
    w2tp = np.zeros((NHP, NO), np.float32)
    w2tp[:NH] = W2.T
    w2s = w2tp.reshape(NT, 128, NO).transpose(1, 0, 2)  # [128, NT, NO]

    base = np.zeros((128, BLOBF), np.float32)
    base[:NI, OFF_W1 : OFF_W1 + NHP] = w1t
    base[:, OFF_B1 : OFF_B1 + NT] = b1c
    base[:, OFF_W2 : OFF_W2 + NT * NO] = w2s.reshape(128, NT * NO)
    base[:, OFF_B2 : OFF_B2 + NBLK * NO] = np.tile(b2, NBLK)[None, :]
    base[:, OFF_ID : OFF_ID + 128] = np.eye(128, dtype=np.float32)

    xs = x.reshape(NCORES, BS, NI)
    in_maps = []
    for i in range(NCORES):
        blob = base.copy()
        blob[:NI, OFF_XT : OFF_XT + BS] = xs[i].T
        in_maps.append({"blob": blob})
    return in_maps


def _run(x, W1, b1, W2, b2, **spmd_kwargs):
    nc = _get_prog()
    in_maps = _make_in_maps(x, W1, b1, W2, b2)
    res = run_bass_kernel_spmd(nc, in_maps, list(range(NCORES)), **spmd_kwargs)
    m2s, s2s = [], []
    for i in range(NCORES):
        m2 = res.results[i]["m2"].reshape(128, T, NBLK, NO)
        s2 = res.results[i]["s2"].reshape(128, T, NBLK, NO)
        # [p, t, blk, o] -> [t, blk*128+p, o]
        m2s.append(m2.transpose(1, 2, 0, 3).reshape(T, BS, NO))
        s2s.append(s2.transpose(1, 2, 0, 3).reshape(T, BS, NO))
    m2 = np.concatenate(m2s, axis=1)
    s2 = np.concatenate(s2s, axis=1)
    return (np.asarray(m2, np.float32), np.asarray(s2, np.float32)), res


def kernel(x, W1, b1, W2, b2):
    out, _ = _run(x, W1, b1, W2, b2)
    return out

